# revision 41
# baseline (speedup 1.0000x reference)
"""GQA attention (32 heads, 8 KV groups, rope, causal) on 8 TRN2 NeuronCores.

Sharding: tensor-parallel over KV groups — core g owns KV group g
(4 query heads + 1 kv head). Wq/Wk/Wv sharded column-wise, Wo row-wise;
each core produces a partial transposed output outT=[D,T] in bf16,
summed in fp32 and transposed on the host.

Per-core dataflow (T=2048 tokens, D=4096, head_dim=128), all matmuls
bf16 with fp32 PSUM:
  proj:  pair-psums q01/q23/kv accumulate over 32 k-tiles; psum halves
         are ACT-evacuated to bf16 and rope runs as 4 bf16 DVE ops.
         v is PE-transposed (4 blocks into one packed psum) to tokens-major.
  attn (chunk I = 512 queries, 2 heads at a time, software-pipelined):
         S-pair[j, i|i'] = kt_J @ (q_h0|q_h1)  -> masked adds (DVE, width-
         restricted) -> one exp (ACT) -> pt pair (bf16)
         ctx-pair += v_J^T @ pt halves;  rowsums via ones-matmuls into one
         psum bank at partition 0 / 32 (col-group pair).
         Normalization: batched DVE reciprocal of the 4 rowsum rows per
         chunk, gpsimd partition-broadcast, DVE multiply.
  out:   resident wo (bf16) stationary tiles; paired [128,1024] psum,
         paired copies and 2KB-line DMAs.
DMA queues: x + out on sync HWDGE, weights (wq/wk/wv/wo) on gpsimd SWDGE,
constants on scalar HWDGE.
"""
import math

import ml_dtypes
import numpy as np

import concourse.bass as bass
import concourse.tile as tile
from concourse import bacc, mybir
from concourse.bass_utils import run_bass_kernel_spmd
from concourse.masks import make_identity

F32 = mybir.dt.float32
BF16 = mybir.dt.bfloat16
NPBF16 = ml_dtypes.bfloat16

T = 2048          # tokens
D = 4096          # model dim
HD = 128          # head dim
NH = 4            # heads per core
DQ = NH * HD      # 512 q dims per core
TC = 512          # token chunk (psum free dim)
NCH = T // TC     # 4 chunks
KT = D // 128     # 32 contraction tiles
JT = T // 128     # 16 key tiles
NET = D // 128    # 32 output-row tiles (of outT)
SCALE = 1.0 / math.sqrt(HD)
NCORES = 8
EXPF = mybir.ActivationFunctionType.Exp


def build_nc():
    nc = bacc.Bacc("TRN2", target_bir_lowering=False, debug=False, num_devices=NCORES)
    xT = nc.dram_tensor("xT", [D, T], BF16, kind="ExternalInput").ap()
    wq = nc.dram_tensor("wq", [128, KT * DQ], BF16, kind="ExternalInput").ap()
    wk = nc.dram_tensor("wk", [128, KT * HD], BF16, kind="ExternalInput").ap()
    wv = nc.dram_tensor("wv", [128, KT * HD], BF16, kind="ExternalInput").ap()
    wo = nc.dram_tensor("wo", [128, NET * DQ], BF16, kind="ExternalInput").ap()
    cosT = nc.dram_tensor("cosT", [HD, T], BF16, kind="ExternalInput").ap()
    sinT = nc.dram_tensor("sinT", [HD, T], BF16, kind="ExternalInput").ap()
    ones = nc.dram_tensor("ones", [128, 2], BF16, kind="ExternalInput").ap()
    out = nc.dram_tensor("out", [D, T], BF16, kind="ExternalOutput").ap()

    with tile.TileContext(nc) as tc:
        _body(tc, out, xT, wq, wk, wv, wo, cosT, sinT, ones)
    nc.compile()
    return nc


def _body(tc, out, xT, wq, wk, wv, wo, cosT, sinT, ones):
    nc = tc.nc
    from contextlib import ExitStack

    with ExitStack() as ctx:
        const_pool = ctx.enter_context(tc.tile_pool(name="const", bufs=1))
        w_pool = ctx.enter_context(tc.tile_pool(name="wp", bufs=1))
        x_pool = ctx.enter_context(tc.tile_pool(name="xp", bufs=6))
        qt_pool = ctx.enter_context(tc.tile_pool(name="qtp", bufs=4))
        kt_pool = ctx.enter_context(tc.tile_pool(name="ktp", bufs=4))
        v4_pool = ctx.enter_context(tc.tile_pool(name="v4p", bufs=4))
        vt_pool = ctx.enter_context(tc.tile_pool(name="vtp", bufs=1))
        pt_pool = ctx.enter_context(tc.tile_pool(name="ptp", bufs=3))
        cx_pool = ctx.enter_context(tc.tile_pool(name="cxp", bufs=16))
        rope_pool = ctx.enter_context(tc.tile_pool(name="ropep", bufs=2))
        rb_pool = ctx.enter_context(tc.tile_pool(name="rbp", bufs=2))
        sr_pool = ctx.enter_context(tc.tile_pool(name="srp", bufs=2))
        o_pool = ctx.enter_context(tc.tile_pool(name="op", bufs=4))
        cs_pool = ctx.enter_context(tc.tile_pool(name="csp", bufs=2))
        ps_pool = ctx.enter_context(tc.tile_pool(name="ps", bufs=3, space="PSUM"))

        # ---- constants (scalar HWDGE queue) ----
        ones_sb = const_pool.tile([128, 2], BF16, tag="ones")
        ident_sb = const_pool.tile([128, 128], BF16, tag="ident")
        nc.scalar.dma_start(ones_sb[:], ones[:, :])

        # ---- resident weights (bf16, partition-major host layout) on the
        # gpsimd SWDGE queue so they never contend with x on sync ----
        wq_sb = w_pool.tile([128, KT * DQ], BF16, tag="wq")
        wk_sb = w_pool.tile([128, KT * HD], BF16, tag="wk")
        wv_sb = w_pool.tile([128, KT * HD], BF16, tag="wv")
        wo_sb = w_pool.tile([128, NET * DQ], BF16, tag="wo")
        for g in range(8):  # 4 k-tiles per transfer; wq on the gpsimd SWDGE
            nc.gpsimd.dma_start(  # queue, wk/wv on the scalar HWDGE queue so
                wq_sb[:, g * 4 * DQ:(g + 1) * 4 * DQ],  # they load in parallel
                wq[:, g * 4 * DQ:(g + 1) * 4 * DQ],
            )
            nc.scalar.dma_start(
                wk_sb[:, g * 4 * HD:(g + 1) * 4 * HD],
                wk[:, g * 4 * HD:(g + 1) * 4 * HD],
            )
            nc.scalar.dma_start(
                wv_sb[:, g * 4 * HD:(g + 1) * 4 * HD],
                wv[:, g * 4 * HD:(g + 1) * 4 * HD],
            )
        # after the weight DMAs: the gpsimd library load this triggers would
        # otherwise delay the whole SWDGE queue at startup
        make_identity(nc, ident_sb[:])

        kt_tiles = []      # kT chunk tiles [128, TC] (d x tokens), bf16
        v4_tiles = []      # packed vT tiles [128, TC] (tokens x d), bf16
        cx_tiles = {}      # (h, chunk) -> ctxT tile [128, TC], bf16

        # per-chunk normalization (reciprocal + broadcast + scale) is
        # deferred past the NEXT chunk's rope emission so its DVE work never
        # sits ahead of the ropes in the FIFO
        norm_jobs = []

        def flush_norm():
            while norm_jobs:
                I0, srh0, cxu0 = norm_jobs.pop(0)
                for h in range(NH):
                    sl = srh0[h]
                    nc.vector.reciprocal_approx_fast(sl[:], sl[:])
                    rb = rb_pool.tile([128, TC], F32, tag="rb", bufs=4,
                                      name=f"rb_{I0}_{h}")
                    nc.gpsimd.partition_broadcast(rb[:], sl[:])
                    cxt = cxu0[h]
                    nc.vector.tensor_mul(cxt[:], cxt[:], rb[:])
                    cx_tiles[(h, I0)] = cxt

        for c in range(NCH):
            # ================= projections for token chunk c =================
            ps_q01 = ps_pool.tile([128, 2 * TC], F32, tag="pair",
                                  name=f"psq01_{c}")
            ps_q23 = ps_pool.tile([128, 2 * TC], F32, tag="pair",
                                  name=f"psq23_{c}")
            ps_kv = ps_pool.tile([128, 2 * TC], F32, tag="pair",
                                 name=f"pskv_{c}")
            for k in range(KT):
                xt = x_pool.tile([128, TC], BF16, tag="x", name=f"x_{c}_{k}")
                nc.sync.dma_start(
                    xt[:], xT[k * 128:(k + 1) * 128, c * TC:(c + 1) * TC]
                )
                first, last = k == 0, k == KT - 1
                for h in range(NH):
                    dst = ps_q01 if h < 2 else ps_q23
                    nc.tensor.matmul(
                        dst[:, (h % 2) * TC:(h % 2 + 1) * TC],
                        wq_sb[:, k * DQ + h * HD:k * DQ + (h + 1) * HD],
                        xt[:],
                        start=first, stop=last,
                    )
                nc.tensor.matmul(
                    ps_kv[:, 0:TC], wk_sb[:, k * HD:(k + 1) * HD], xt[:],
                    start=first, stop=last,
                )
                nc.tensor.matmul(
                    ps_kv[:, TC:2 * TC], wv_sb[:, k * HD:(k + 1) * HD], xt[:],
                    start=first, stop=last,
                )

            if c == 0:  # wo after the chunk-0 weights on the same queue
                for g in range(4):
                    nc.gpsimd.dma_start(
                        wo_sb[:, g * 8 * DQ:(g + 1) * 8 * DQ],
                        wo[:, g * 8 * DQ:(g + 1) * 8 * DQ],
                    )

            cs_t = cs_pool.tile([HD, TC], BF16, tag="cos", name=f"cos_{c}")
            sn_t = cs_pool.tile([HD, TC], BF16, tag="sin", name=f"sin_{c}")
            nc.scalar.dma_start(cs_t[:], cosT[:, c * TC:(c + 1) * TC])
            nc.scalar.dma_start(sn_t[:], sinT[:, c * TC:(c + 1) * TC])
            cs = cs_t[:, :]
            sn = sn_t[:, :]

            def rope(ps_half, dst_pool, tag, nm):
                # psum-direct muls (crossed reads must come from PSUM — the
                # verifier requires SBUF operands to share start partitions),
                # bf16 outputs so the final add runs in the 2x DVE mode
                t1 = rope_pool.tile([128, TC], BF16, tag="t1", name=f"r1{nm}")
                t2 = rope_pool.tile([128, TC], BF16, tag="t2", name=f"r2{nm}")
                nc.vector.tensor_mul(t2[0:64, :], ps_half[64:128, :],
                                     sn[0:64, :])
                nc.vector.tensor_mul(t2[64:128, :], ps_half[0:64, :],
                                     sn[64:128, :])
                nc.vector.tensor_mul(t1[:], ps_half, cs)
                d = dst_pool.tile([128, TC], BF16, tag=tag, name=nm)
                nc.vector.tensor_add(d[:], t1[:], t2[:])
                return d

            # v-half evacuation first so the PE transposes aren't gated by
            # the rope chain on the DVE; then the ropes in need order
            # (chunk 0's attention needs kt immediately; later chunks start
            # on old kt tiles and need q0/q1 first)
            vt = vt_pool.tile([128, TC], BF16, tag="vt", name=f"vt_{c}")
            nc.vector.tensor_copy(vt[:], ps_kv[:, TC:2 * TC])
            q_chunk = [None] * NH
            if c == 0:
                kt = rope(ps_kv[:, 0:TC], kt_pool, "kt", f"kt_{c}")
                q_chunk[0] = rope(ps_q01[:, 0:TC], qt_pool, "qt", f"qt_{c}_0")
                q_chunk[1] = rope(ps_q01[:, TC:2 * TC], qt_pool, "qt",
                                  f"qt_{c}_1")
            else:
                q_chunk[0] = rope(ps_q01[:, 0:TC], qt_pool, "qt", f"qt_{c}_0")
                q_chunk[1] = rope(ps_q01[:, TC:2 * TC], qt_pool, "qt",
                                  f"qt_{c}_1")
                kt = rope(ps_kv[:, 0:TC], kt_pool, "kt", f"kt_{c}")
            kt_tiles.append(kt)

            ps_t = ps_pool.tile([128, TC], BF16, tag="pair",
                                name=f"pst_{c}")
            for jj in range(TC // 128):
                nc.tensor.transpose(ps_t[:, jj * 128:(jj + 1) * 128],
                                    vt[:, jj * 128:(jj + 1) * 128],
                                    ident_sb[:])
            v4 = v4_pool.tile([128, TC], BF16, tag="v4", name=f"v4_{c}")
            nc.vector.tensor_copy(v4[:], ps_t[:])
            v4_tiles.append(v4)

            q_chunk[2] = rope(ps_q23[:, 0:TC], qt_pool, "qt", f"qt_{c}_2")
            q_chunk[3] = rope(ps_q23[:, TC:2 * TC], qt_pool, "qt", f"qt_{c}_3")
            flush_norm()

            # ========== attention for i-chunk I = c, two heads at a time =====
            # Each head's rowsum accumulation group gets its OWN psum bank
            # (start=True clears has_written state per bank; sharing a bank
            # between groups accumulates onto stale data).
            I = c
            nj = 4 * I + 4
            ctx_un = {}
            srh = {}
            for hp in range(NH // 2):
                h0, h1 = 2 * hp, 2 * hp + 1
                ps_ctx = ps_pool.tile([128, 2 * TC], F32, tag="pair",
                                      name=f"psctx_{I}_{hp}")
                # per-head rowsum groups in their OWN banks (start=True
                # clears has_written state; groups must not share a bank)
                ps_s0 = ps_pool.tile([2, TC], F32, tag="one", bufs=2,
                                     name=f"pssum_{I}_{h0}")
                ps_s1 = ps_pool.tile([2, TC], F32, tag="one", bufs=2,
                                     name=f"pssum_{I}_{h1}")
                pts = {}

                def ctx_ones(J):
                    # Fully-masked i-subtiles (i-block < q) are skipped, so
                    # diagonal tiles write only cols [q*128, TC). Per column
                    # subtile s the last writer is diagonal J = 4I + s, which
                    # must carry its stop flag — hence the split matmuls.
                    first = J == 0
                    q = J - 4 * I
                    pt2 = pts.pop(J)
                    vst = v4_tiles[J // 4][:, (J % 4) * 128:(J % 4 + 1) * 128]
                    if q < 0:  # off-diagonal: full width, never a last writer
                        nc.tensor.matmul(ps_ctx[:, 0:TC], vst, pt2[:, 0:TC],
                                         start=first, stop=False)
                        nc.tensor.matmul(ps_ctx[:, TC:2 * TC], vst,
                                         pt2[:, TC:2 * TC],
                                         start=first, stop=False)
                        nc.tensor.matmul(ps_s0[:], ones_sb[:], pt2[:, 0:TC],
                                         start=first, stop=False)
                        nc.tensor.matmul(ps_s1[:], ones_sb[:],
                                         pt2[:, TC:2 * TC],
                                         start=first, stop=False)
                        return
                    w0, w1 = q * 128, (q + 1) * 128
                    for base in (0, TC):
                        nc.tensor.matmul(ps_ctx[:, base + w0:base + w1], vst,
                                         pt2[:, base + w0:base + w1],
                                         start=first, stop=True)
                        if w1 < TC:
                            nc.tensor.matmul(ps_ctx[:, base + w1:base + TC],
                                             vst, pt2[:, base + w1:base + TC],
                                             start=first, stop=False)
                    for ps_s, base in ((ps_s0, 0), (ps_s1, TC)):
                        nc.tensor.matmul(ps_s[:, w0:w1], ones_sb[:],
                                         pt2[:, base + w0:base + w1],
                                         start=first, stop=True)
                        if w1 < TC:
                            nc.tensor.matmul(ps_s[:, w1:TC], ones_sb[:],
                                             pt2[:, base + w1:base + TC],
                                             start=first, stop=False)

                for J in range(nj):
                    s2 = ps_pool.tile([128, 2 * TC], F32, tag="pair",
                                      name=f"pss_{I}_{hp}_{J}")
                    kst = kt_tiles[J // 4][:, (J % 4) * 128:(J % 4 + 1) * 128]
                    q = J - 4 * I
                    if q < 0:  # off-diagonal: full query range
                        nc.tensor.matmul(s2[:, 0:TC], kst, q_chunk[h0][:],
                                         start=True, stop=True)
                        nc.tensor.matmul(s2[:, TC:2 * TC], kst,
                                         q_chunk[h1][:],
                                         start=True, stop=True)
                        e0 = 0
                    else:
                        # diagonal: skip fully-masked i-subtiles; the causal
                        # triangle is zeroed on pt AFTER the exp (gpsimd
                        # affine_select — keeps the DVE out of the S->exp
                        # chain)
                        w0 = q * 128
                        nc.tensor.matmul(s2[:, w0:TC], kst,
                                         q_chunk[h0][:, w0:TC],
                                         start=True, stop=True)
                        nc.tensor.matmul(s2[:, TC + w0:2 * TC], kst,
                                         q_chunk[h1][:, w0:TC],
                                         start=True, stop=True)
                        e0 = w0
                    pt2 = pt_pool.tile([128, 2 * TC], BF16, tag="pt",
                                       name=f"pt_{I}_{hp}_{J}")
                    nc.scalar.activation(pt2[:, e0:2 * TC], s2[:, e0:2 * TC],
                                         EXPF, scale=SCALE)
                    if q >= 0:
                        # zero pt where j > i: iota = c - p, keep when >= 0
                        for base in (w0, TC + w0):
                            nc.gpsimd.affine_select(
                                out=pt2[:, base:base + 128],
                                in_=pt2[:, base:base + 128],
                                compare_op=mybir.AluOpType.is_ge,
                                fill=0.0,
                                base=0,
                                pattern=[[1, 128]],
                                channel_multiplier=-1,
                            )
                    pts[J] = pt2
                    if J >= 1:
                        ctx_ones(J - 1)
                ctx_ones(nj - 1)

                # evacuate: ctx halves first (they gate the next chunk's
                # psum slots), then rowsum copies + fast [1,TC] reciprocals
                cx0 = cx_pool.tile([128, TC], BF16, tag="cx",
                                   name=f"cx_{I}_{h0}")
                nc.vector.tensor_copy(cx0[:], ps_ctx[:, 0:TC])
                ctx_un[h0] = cx0
                cx1 = cx_pool.tile([128, TC], BF16, tag="cx",
                                   name=f"cx_{I}_{h1}")
                nc.vector.tensor_copy(cx1[:], ps_ctx[:, TC:2 * TC])
                ctx_un[h1] = cx1
                for h, ps_s in ((h0, ps_s0), (h1, ps_s1)):
                    sl = sr_pool.tile([1, TC], F32, tag="sl", bufs=8,
                                      name=f"sl_{I}_{h}")
                    nc.scalar.copy(sl[:], ps_s[0:1, :])
                    srh[h] = sl

            norm_jobs.append((I, dict(srh), dict(ctx_un)))

        flush_norm()

        # ======= output stage: outT[e,t], resident wo stationary tiles =======
        for Et in range(NET):
            ps_o = [ps_pool.tile([128, 2 * TC], F32, tag="pair",
                                 name=f"pso_{Et}_{p}") for p in range(2)]
            for h in range(NH):
                wst = wo_sb[:, Et * DQ + h * HD:Et * DQ + (h + 1) * HD]
                for tc_ in range(NCH):
                    nc.tensor.matmul(
                        ps_o[tc_ // 2][:, (tc_ % 2) * TC:(tc_ % 2 + 1) * TC],
                        wst,
                        cx_tiles[(h, tc_)][:],
                        start=h == 0, stop=h == NH - 1,
                    )
            for p in range(2):
                ot = o_pool.tile([128, 2 * TC], BF16, tag="o",
                                 name=f"o_{Et}_{p}")
                if p == 0:
                    nc.vector.tensor_copy(ot[:], ps_o[p][:])
                else:
                    nc.scalar.copy(ot[:], ps_o[p][:])
                nc.sync.dma_start(
                    out[Et * 128:(Et + 1) * 128,
                        p * 2 * TC:(p + 1) * 2 * TC],
                    ot[:],
                )


# ---------------------------------------------------------------------------
# host side
# ---------------------------------------------------------------------------
_NC_CACHE = None


def _get_nc():
    global _NC_CACHE
    if _NC_CACHE is None:
        _NC_CACHE = build_nc()
    return _NC_CACHE


def _pmajor(w, kt, width):
    """[kt*128, width] -> partition-major [128, kt*width] bf16."""
    return np.ascontiguousarray(
        w.reshape(kt, 128, width).transpose(1, 0, 2).reshape(128, kt * width)
    )


def make_in_maps(x, Wq, Wk, Wv, Wo, cos, sin):
    x = np.asarray(x, dtype=np.float32)
    xT = np.ascontiguousarray(x.reshape(T, D).T.astype(NPBF16))
    cosT = np.ascontiguousarray(
        np.asarray(cos, np.float32)[:T].T.astype(NPBF16))
    sin_t = np.asarray(sin, np.float32)[:T]          # [T, 128]
    sinT = sin_t.T.copy()                            # [128, T]
    sinT[:64] *= -1.0                                # fold rotate-half sign
    sinT = np.ascontiguousarray(sinT.astype(NPBF16))

    ones = np.ones((128, 2), NPBF16)

    Wq = np.asarray(Wq, np.float32).astype(NPBF16)
    Wk = np.asarray(Wk, np.float32).astype(NPBF16)
    Wv = np.asarray(Wv, np.float32).astype(NPBF16)
    Wo = np.asarray(Wo, np.float32).astype(NPBF16)
    in_maps = []
    for g in range(NCORES):
        # wo rows [g*DQ:(g+1)*DQ] shuffled to [dh, (Et, h, e)] so Et-tiles are
        # resident stationary slices of one partition-major tensor
        w = Wo[g * DQ:(g + 1) * DQ, :]                          # [512, 4096]
        w4 = w.reshape(NH, HD, NET, 128).transpose(1, 2, 0, 3)  # [dh,Et,h,e]
        woP = np.ascontiguousarray(w4.reshape(128, NET * DQ))
        in_maps.append({
            "xT": xT,
            "wq": _pmajor(Wq[:, g * DQ:(g + 1) * DQ], KT, DQ),
            "wk": _pmajor(Wk[:, g * HD:(g + 1) * HD], KT, HD),
            "wv": _pmajor(Wv[:, g * HD:(g + 1) * HD], KT, HD),
            "wo": woP,
            "cosT": cosT,
            "sinT": sinT,
            "ones": ones,
        })
    return in_maps


def kernel(x, Wq, Wk, Wv, Wo, cos, sin):
    nc = _get_nc()
    in_maps = make_in_maps(x, Wq, Wk, Wv, Wo, cos, sin)
    res = run_bass_kernel_spmd(nc, in_maps, core_ids=list(range(NCORES)))
    acc = np.zeros((D, T), np.float32)
    for c in range(NCORES):
        acc += res.results[c]["out"].astype(np.float32)
    return np.ascontiguousarray(acc.T).reshape(1, T, D)
